# revision 37
# baseline (speedup 1.0000x reference)
"""Trainium2 Bass kernel for causal multi-head attention (prefill), v9 final.

Problem: x[2,2048,768], 12 heads x 64 dim, causal softmax(QK^T/8)V + out-proj.

Sharding (8 cores, no collectives): core c handles batch c//4 and head group
c%4 (3 heads).  Host sums the 4 partial outputs per batch and transposes.

Key design points (vs the 290us v1 baseline):
- everything bf16 (half DMA traffic, FWL-eligible weight loads); PSUM fp32.
- packed Q+K projection into 3 full 128-row tiles; per-head q/k pairs share a
  base partition (k_h2 re-copied to base 0 during PSUM evacuation).
- softmax denominator: ctx weights are [v_h | ones], so PSUM rows 64:128
  accumulate the denominator replicated 64x for free; normalization uses the
  fast custom-DVE reciprocal (fp32, SBUF-only -> one bounce copy).
- engine queues are strict FIFO, so projection / out-projection work is
  SMEARED into the attention chains one unit per few kv-steps instead of
  batched between chains (batching head-of-line blocks the PE queue and
  starves the scalar engine's exp stream).
"""

import numpy as np
import ml_dtypes

import concourse.bass as bass
import concourse.tile as tile
from concourse import bacc, mybir
from concourse.bass_utils import run_bass_kernel_spmd

F32 = mybir.dt.float32
BF = mybir.dt.bfloat16

B, S, D = 2, 2048, 768
H, DH = 12, 64
HPC = 3                 # heads per core
GH = HPC * DH           # 192 head dims per core
NCORES = 8
KT = D // 128           # 6 contraction tiles for projections
NKV = S // 128          # 16 kv tiles of 128
WJ = 1024               # attention q-window width
NJ = S // WJ            # 2 windows

# tuning knobs (hw-validated; base = the measured-best v2 structure)
OPT = {
    "head_order": (0, 1, 2),   # per-window head order
    "defer": 0,                # ctx lags scores/exp by this many kv-steps
    "smear": 3,                # emit 1 filler unit per this many attn steps
    "store_split": False,      # split outproj stores into 2x[128,512]
    "shift_split": True,       # split h1 partition-shift into 2 DMAs
    "dma_scalar": False,       # issue half the input DMAs from scalar queue
    "esb_bufs": 3,
}


def build(opt=None):
    o = dict(OPT)
    if opt:
        o.update(opt)
    nc = bacc.Bacc("TRN2", target_bir_lowering=False, debug=False)

    xT = nc.dram_tensor("xT", [D, S], BF, kind="ExternalInput")
    wqk = nc.dram_tensor("wqk", [D, 2 * GH], BF, kind="ExternalInput")
    wv = nc.dram_tensor("wv", [D, GH], BF, kind="ExternalInput")
    wo = nc.dram_tensor("wo", [GH, D], BF, kind="ExternalInput")
    tri = nc.dram_tensor("tri", [128, 128], BF, kind="ExternalInput")
    outT = nc.dram_tensor("outT", [D, S], BF, kind="ExternalOutput")

    with tile.TileContext(nc) as tc, \
         nc.allow_low_precision(reason="bf16 compute, fp32 accumulation"):
        with tc.tile_pool(name="sb", bufs=1) as sb, \
             tc.tile_pool(name="sbe", bufs=o["esb_bufs"]) as sbe, \
             tc.tile_pool(name="sbo", bufs=2) as sbo, \
             tc.tile_pool(name="ps", bufs=o.get("sc_bufs", 2),
                          space="PSUM") as ps, \
             tc.tile_pool(name="psc", bufs=o.get("ctx_bufs", 2),
                          space="PSUM") as psc:

            # ---- phase 0: loads ----
            eng2 = nc.scalar if o["dma_scalar"] else nc.sync
            wqk_sb = sb.tile([128, KT, 2 * GH], BF, tag="wqk")
            wqk_r = wqk[:, :].rearrange("(k p) m -> p k m", p=128)
            for m in range(3):
                nc.sync.dma_start(wqk_sb[:, :, m * 128:(m + 1) * 128],
                                  wqk_r[:, :, m * 128:(m + 1) * 128])
            wv_sb = sb.tile([128, KT, GH], BF, tag="wv")
            eng2.dma_start(wv_sb, wv[:, :].rearrange("(k p) m -> p k m", p=128))

            xsb = sb.tile([128, KT, S], BF, tag="xsb")
            x_r = xT[:, :].rearrange("(k p) n -> p k n", p=128)
            for k in range(KT):
                eng = nc.sync if k % 2 == 0 else eng2
                for half in range(2):
                    c0 = half * 512
                    eng.dma_start(xsb[:, k, c0:c0 + 512],
                                  x_r[:, k, c0:c0 + 512])
            for k in range(KT):
                eng = nc.sync if k % 2 == 0 else eng2
                eng.dma_start(xsb[:, k, WJ:S], x_r[:, k, WJ:S])

            tri_sb = sb.tile([128, 128], BF, tag="tri")
            eng2.dma_start(tri_sb, tri[:, :])
            wo01_sb = sb.tile([128, D], BF, tag="wo01")
            wo2_sb = sb.tile([64, D], BF, tag="wo2")
            eng2.dma_start(wo01_sb, wo[0:128, :])
            eng2.dma_start(wo2_sb, wo[128:GH, :])

            # ---- persistent sbuf tensors ----
            qkT = sb.tile([128, 3, S], BF, tag="qkT")
            kh2b = sb.tile([64, S], BF, tag="kh2b")
            vaug = sb.tile([128, NKV, HPC, 128], BF, tag="vaug")
            nc.vector.memset(vaug[:, :, :, 64:128], 1.0)
            ctxT01 = sb.tile([128, S], BF, tag="ctxT01")
            ctxT2 = sb.tile([64, S], BF, tag="ctxT2")

            def qslc(h, c0, c1):
                return (qkT[0:64, 0, c0:c1], qkT[64:128, 0, c0:c1],
                        qkT[0:64, 2, c0:c1])[h]

            def kslc(h, i):
                c0, c1 = i * 128, (i + 1) * 128
                return (qkT[0:64, 1, c0:c1], qkT[64:128, 1, c0:c1],
                        kh2b[:, c0:c1])[h]

            def proj_qk(mt, ch, half=None):
                # half=None: both 512-col blocks; 0/1: that block only (so
                # filler units stay small enough not to gap the exp stream)
                c0 = ch * WJ
                halves = (0, 1) if half is None else (half,)
                pp = ps.tile([128, WJ], F32, tag="sc", name="pp")
                for nb in halves:
                    s0 = nb * 512
                    for k in range(KT):
                        nc.tensor.matmul(
                            pp[:, s0:s0 + 512],
                            wqk_sb[:, k, mt * 128:(mt + 1) * 128],
                            xsb[:, k, c0 + s0:c0 + s0 + 512],
                            start=(k == 0), stop=(k == KT - 1))
                    d0, d1 = c0 + s0, c0 + s0 + 512
                    if mt < 2:
                        nc.vector.tensor_copy(qkT[:, mt, d0:d1],
                                              pp[:, s0:s0 + 512])
                    else:
                        nc.vector.tensor_copy(qkT[0:64, 2, d0:d1],
                                              pp[0:64, s0:s0 + 512])
                        nc.vector.tensor_copy(kh2b[:, d0:d1],
                                              pp[64:128, s0:s0 + 512])

            def proj_v(i):
                pp = ps.tile([128, WJ], F32, tag="sc", name="pp")
                for k in range(KT):
                    nc.tensor.matmul(
                        pp[:, 0:GH],
                        xsb[:, k, i * 128:(i + 1) * 128],
                        wv_sb[:, k, :],
                        start=(k == 0), stop=(k == KT - 1))
                nc.vector.tensor_copy(
                    vaug[:, i, :, 0:64],
                    pp[:, 0:GH].rearrange("p (h c) -> p h c", c=64))

            def attn_front(J, h, i):
                d = 128 * i - WJ * J   # window col where the diagonal starts
                col0 = max(0, d)
                nb0 = max(0, d // 512)
                spsum = ps.tile([128, WJ], F32, tag="sc", name="spsum")
                for nb in range(nb0, 2):
                    s0 = max(nb * 512, col0)
                    nc.tensor.matmul(
                        spsum[:, s0:(nb + 1) * 512],
                        kslc(h, i),
                        qslc(h, WJ * J + s0, WJ * J + (nb + 1) * 512),
                        start=True, stop=True)
                esb = sbe.tile([128, WJ], BF, tag="exp", name="esb")
                nc.scalar.activation(
                    esb[:, col0:WJ], spsum[:, col0:WJ],
                    mybir.ActivationFunctionType.Exp, scale=0.125)
                if d >= 0:
                    nc.vector.tensor_mul(
                        esb[:, d:d + 128], esb[:, d:d + 128], tri_sb)
                return esb

            def attn_back(J, h, i, ctx_ps, esb):
                d = 128 * i - WJ * J
                col0 = max(0, d)
                nb0 = max(0, d // 512)
                for nb in range(nb0, 2):
                    s0 = max(nb * 512, col0)
                    nc.tensor.matmul(
                        ctx_ps[:, s0:(nb + 1) * 512],
                        vaug[:, i, h, :],
                        esb[:, s0:(nb + 1) * 512],
                        start=(i == 0), stop=(i == 8 * J + 4 * nb + 3))

            def attn_norm(J, h, ctx_ps):
                den = sbo.tile([64, WJ], F32, tag="den", name="den")
                nc.vector.tensor_copy(den, ctx_ps[64:128, :])
                inv = sbo.tile([64, WJ], F32, tag="inv", name="inv")
                nc.vector.reciprocal_approx_fast(inv, den)
                h1tmp = (sbo.tile([64, WJ], BF, tag="h1tmp", name="h1tmp")
                         if h == 1 else None)
                dst = (ctxT01[0:64, WJ * J:WJ * (J + 1)], h1tmp,
                       ctxT2[:, WJ * J:WJ * (J + 1)])[h]
                nc.vector.tensor_mul(dst, ctx_ps[0:64, :], inv)
                if h == 1:
                    if o["shift_split"]:
                        for half in range(2):
                            c0 = half * 512
                            nc.sync.dma_start(
                                ctxT01[64:128, WJ * J + c0:WJ * J + c0 + 512],
                                h1tmp[:, c0:c0 + 512])
                    else:
                        nc.sync.dma_start(
                            ctxT01[64:128, WJ * J:WJ * (J + 1)], h1tmp)

            def out_proj(J, mt):
                ops = psc.tile([128, WJ], F32, tag="ctx", name="ops")
                for jj in range(2):
                    c0 = WJ * J + jj * 512
                    nc.tensor.matmul(
                        ops[:, jj * 512:(jj + 1) * 512],
                        wo01_sb[:, mt * 128:(mt + 1) * 128],
                        ctxT01[:, c0:c0 + 512], start=True, stop=False)
                    nc.tensor.matmul(
                        ops[:, jj * 512:(jj + 1) * 512],
                        wo2_sb[:, mt * 128:(mt + 1) * 128],
                        ctxT2[:, c0:c0 + 512], start=False, stop=True)
                if o["store_split"] or J == 1:
                    for jj in range(2):
                        osb = sbo.tile([128, 512], BF, tag="osb", name="osb")
                        if J == 1 and jj == 1:
                            # scalar engine is idle at the tail (exps done):
                            # split the PSUM evacuation across ACT + DVE
                            nc.scalar.copy(osb,
                                           ops[:, jj * 512:(jj + 1) * 512])
                        else:
                            nc.vector.tensor_copy(
                                osb, ops[:, jj * 512:(jj + 1) * 512])
                        nc.sync.dma_start(
                            outT[mt * 128:(mt + 1) * 128,
                                 WJ * J + jj * 512:WJ * J + (jj + 1) * 512],
                            osb)
                else:
                    osb = sbo.tile([128, WJ], BF, tag="osb", name="osb")
                    nc.vector.tensor_copy(osb, ops)
                    nc.sync.dma_start(
                        outT[mt * 128:(mt + 1) * 128, WJ * J:WJ * (J + 1)],
                        osb)

            def attention(J, h, fillers, smear=None, pops=1):
                # fillers: deque of closures to smear between kv-steps;
                # every `smear` steps, pop `pops` units
                smear = o["smear"] if smear is None else smear
                ctx_ps = psc.tile([128, WJ], F32, tag="ctx", name="ctx_ps")
                n = 8 * J + 8
                fifo = []
                for i in range(n):
                    esb = attn_front(J, h, i)
                    fifo.append((J, h, i, ctx_ps, esb))
                    if len(fifo) > o["defer"]:
                        attn_back(*fifo.pop(0))
                    if smear and i % smear == smear - 1:
                        for _ in range(pops):
                            if fillers:
                                fillers.pop(0)()
                for e in fifo:
                    attn_back(*e)
                attn_norm(J, h, ctx_ps)

            # ---- emission (v2-proven order); out_proj(0) is smeared into
            # attention(1,0) so its DMA-gated matmuls can't head-of-line
            # block the J1 score stream ----
            ho = o["head_order"]
            proj_qk(0, 0)
            proj_qk(1, 0)
            proj_v(0)
            # v_i is only needed at kv-step i -> feed the rest as fillers so
            # the exp stream starts ~10us earlier
            f0 = [lambda i=i: proj_v(i) for i in range(1, 8)]
            f0 += [lambda hf=hf: proj_qk(2, 0, half=hf) for hf in range(2)]
            f0 += [lambda i=i: proj_v(i) for i in range(8, NKV)]
            attention(0, ho[0], f0, smear=1, pops=2)
            while f0:
                f0.pop(0)()
            f1 = []
            for mt in range(3):
                f1 += [lambda mt=mt, hf=hf: proj_qk(mt, 1, half=hf)
                       for hf in range(2)]
            attention(0, ho[1], f1, smear=1)
            while f1:
                f1.pop(0)()
            attention(0, ho[2], [])
            fillers = [lambda mt=mt: out_proj(0, mt) for mt in range(6)]
            attention(1, ho[0], fillers)
            while fillers:
                fillers.pop(0)()
            attention(1, ho[1], [])
            attention(1, ho[2], [])
            for mt in range(6):
                out_proj(1, mt)

    nc.compile()
    return nc


def shard_inputs(x, Wq, Wk, Wv, Wo):
    x = np.asarray(x, np.float32)
    tri = np.triu(np.ones((128, 128), np.float32))

    def bf(a):
        return np.ascontiguousarray(a).astype(ml_dtypes.bfloat16)

    in_maps = []
    for c in range(NCORES):
        b, g = c // 4, c % 4
        rs = slice(GH * g, GH * g + GH)
        Wq_g = np.asarray(Wq, np.float32)[rs]  # [192, 768]
        Wk_g = np.asarray(Wk, np.float32)[rs]
        # packed rows: [q0 q1 | k0 k1 | q2 k2] (64 rows each)
        wqk = np.concatenate(
            [Wq_g[0:128], Wk_g[0:128], Wq_g[128:192], Wk_g[128:192]],
            axis=0).T  # [768, 384]
        in_maps.append({
            "xT": bf(x[b].T),
            "wqk": bf(wqk),
            "wv": bf(np.asarray(Wv, np.float32)[rs].T),
            "wo": bf(np.asarray(Wo, np.float32)[:, rs].T),
            "tri": bf(tri),
        })
    return in_maps


def assemble(results, bo):
    out = np.zeros((B, S, D), np.float32)
    for c in range(NCORES):
        out[c // 4] += results[c]["outT"].astype(np.float32).T
    return out + np.asarray(bo, np.float32)[None, None, :]


_NC = None


def kernel(x, Wq, Wk, Wv, Wo, bo, **run_kwargs):
    global _NC
    if _NC is None:
        _NC = build()
    in_maps = shard_inputs(x, Wq, Wk, Wv, Wo)
    res = run_bass_kernel_spmd(_NC, in_maps, core_ids=list(range(NCORES)),
                               **run_kwargs)
    out = assemble(res.results, bo)
    kernel.last_results = res
    return out


# revision 38
# speedup vs baseline: 1.0458x; 1.0458x over previous
"""Trainium2 Bass kernel for causal multi-head attention (prefill), v9 final.

Problem: x[2,2048,768], 12 heads x 64 dim, causal softmax(QK^T/8)V + out-proj.

Sharding (8 cores, no collectives): core c handles batch c//4 and head group
c%4 (3 heads).  Host sums the 4 partial outputs per batch and transposes.

Key design points (vs the 290us v1 baseline):
- everything bf16 (half DMA traffic, FWL-eligible weight loads); PSUM fp32.
- packed Q+K projection into 3 full 128-row tiles; per-head q/k pairs share a
  base partition (k_h2 re-copied to base 0 during PSUM evacuation).
- softmax denominator: ctx weights are [v_h | ones], so PSUM rows 64:128
  accumulate the denominator replicated 64x for free; normalization uses the
  fast custom-DVE reciprocal (fp32, SBUF-only -> one bounce copy).
- engine queues are strict FIFO, so projection / out-projection work is
  SMEARED into the attention chains one unit per few kv-steps instead of
  batched between chains (batching head-of-line blocks the PE queue and
  starves the scalar engine's exp stream).
"""

import numpy as np
import ml_dtypes

import concourse.bass as bass
import concourse.tile as tile
from concourse import bacc, mybir
from concourse.bass_utils import run_bass_kernel_spmd

F32 = mybir.dt.float32
BF = mybir.dt.bfloat16

B, S, D = 2, 2048, 768
H, DH = 12, 64
HPC = 3                 # heads per core
GH = HPC * DH           # 192 head dims per core
NCORES = 8
KT = D // 128           # 6 contraction tiles for projections
NKV = S // 128          # 16 kv tiles of 128
WJ = 1024               # attention q-window width
NJ = S // WJ            # 2 windows

# tuning knobs (hw-validated; base = the measured-best v2 structure)
OPT = {
    "head_order": (0, 1, 2),   # per-window head order
    "defer": 0,                # ctx lags scores/exp by this many kv-steps
    "smear": 3,                # emit 1 filler unit per this many attn steps
    "store_split": False,      # split outproj stores into 2x[128,512]
    "shift_split": True,       # split h1 partition-shift into 2 DMAs
    "dma_scalar": False,       # issue half the input DMAs from scalar queue
    "esb_bufs": 3,
}


def build(opt=None):
    o = dict(OPT)
    if opt:
        o.update(opt)
    nc = bacc.Bacc("TRN2", target_bir_lowering=False, debug=False)

    xT = nc.dram_tensor("xT", [D, S], BF, kind="ExternalInput")
    wqk = nc.dram_tensor("wqk", [D, 2 * GH], BF, kind="ExternalInput")
    wv = nc.dram_tensor("wv", [D, GH], BF, kind="ExternalInput")
    wo = nc.dram_tensor("wo", [GH, D], BF, kind="ExternalInput")
    tri = nc.dram_tensor("tri", [128, 128], BF, kind="ExternalInput")
    outT = nc.dram_tensor("outT", [D, S], BF, kind="ExternalOutput")

    with tile.TileContext(nc) as tc, \
         nc.allow_low_precision(reason="bf16 compute, fp32 accumulation"):
        with tc.tile_pool(name="sb", bufs=1) as sb, \
             tc.tile_pool(name="sbe", bufs=o["esb_bufs"]) as sbe, \
             tc.tile_pool(name="sbo", bufs=2) as sbo, \
             tc.tile_pool(name="ps", bufs=o.get("sc_bufs", 2),
                          space="PSUM") as ps, \
             tc.tile_pool(name="psc", bufs=o.get("ctx_bufs", 2),
                          space="PSUM") as psc:

            # ---- phase 0: loads ----
            eng2 = nc.scalar if o["dma_scalar"] else nc.sync
            wqk_sb = sb.tile([128, KT, 2 * GH], BF, tag="wqk")
            wqk_r = wqk[:, :].rearrange("(k p) m -> p k m", p=128)
            for m in range(3):
                nc.sync.dma_start(wqk_sb[:, :, m * 128:(m + 1) * 128],
                                  wqk_r[:, :, m * 128:(m + 1) * 128])
            wv_sb = sb.tile([128, KT, GH], BF, tag="wv")
            eng2.dma_start(wv_sb, wv[:, :].rearrange("(k p) m -> p k m", p=128))

            xsb = sb.tile([128, KT, S], BF, tag="xsb")
            x_r = xT[:, :].rearrange("(k p) n -> p k n", p=128)
            for k in range(KT):
                eng = nc.sync if k % 2 == 0 else eng2
                for half in range(2):
                    c0 = half * 512
                    eng.dma_start(xsb[:, k, c0:c0 + 512],
                                  x_r[:, k, c0:c0 + 512])
            for k in range(KT):
                eng = nc.sync if k % 2 == 0 else eng2
                eng.dma_start(xsb[:, k, WJ:S], x_r[:, k, WJ:S])

            tri_sb = sb.tile([128, 128], BF, tag="tri")
            eng2.dma_start(tri_sb, tri[:, :])
            wo01_sb = sb.tile([128, D], BF, tag="wo01")
            wo2_sb = sb.tile([64, D], BF, tag="wo2")
            eng2.dma_start(wo01_sb, wo[0:128, :])
            eng2.dma_start(wo2_sb, wo[128:GH, :])

            # ---- persistent sbuf tensors ----
            qkT = sb.tile([128, 3, S], BF, tag="qkT")
            kh2b = sb.tile([64, S], BF, tag="kh2b")
            vaug = sb.tile([128, NKV, HPC, 128], BF, tag="vaug")
            nc.vector.memset(vaug[:, :, :, 64:128], 1.0)
            ctxT01 = sb.tile([128, S], BF, tag="ctxT01")
            ctxT2 = sb.tile([64, S], BF, tag="ctxT2")

            def qslc(h, c0, c1):
                return (qkT[0:64, 0, c0:c1], qkT[64:128, 0, c0:c1],
                        qkT[0:64, 2, c0:c1])[h]

            def kslc(h, i):
                c0, c1 = i * 128, (i + 1) * 128
                return (qkT[0:64, 1, c0:c1], qkT[64:128, 1, c0:c1],
                        kh2b[:, c0:c1])[h]

            def proj_qk(mt, ch, half=None):
                # half=None: both 512-col blocks; 0/1: that block only (so
                # filler units stay small enough not to gap the exp stream)
                c0 = ch * WJ
                halves = (0, 1) if half is None else (half,)
                pp = ps.tile([128, WJ], F32, tag="sc", name="pp")
                for nb in halves:
                    s0 = nb * 512
                    for k in range(KT):
                        nc.tensor.matmul(
                            pp[:, s0:s0 + 512],
                            wqk_sb[:, k, mt * 128:(mt + 1) * 128],
                            xsb[:, k, c0 + s0:c0 + s0 + 512],
                            start=(k == 0), stop=(k == KT - 1))
                    d0, d1 = c0 + s0, c0 + s0 + 512
                    if mt < 2:
                        nc.vector.tensor_copy(qkT[:, mt, d0:d1],
                                              pp[:, s0:s0 + 512])
                    else:
                        nc.vector.tensor_copy(qkT[0:64, 2, d0:d1],
                                              pp[0:64, s0:s0 + 512])
                        nc.vector.tensor_copy(kh2b[:, d0:d1],
                                              pp[64:128, s0:s0 + 512])

            def proj_v(i):
                pp = ps.tile([128, WJ], F32, tag="sc", name="pp")
                for k in range(KT):
                    nc.tensor.matmul(
                        pp[:, 0:GH],
                        xsb[:, k, i * 128:(i + 1) * 128],
                        wv_sb[:, k, :],
                        start=(k == 0), stop=(k == KT - 1))
                nc.vector.tensor_copy(
                    vaug[:, i, :, 0:64],
                    pp[:, 0:GH].rearrange("p (h c) -> p h c", c=64))

            def attn_front(J, h, i):
                d = 128 * i - WJ * J   # window col where the diagonal starts
                col0 = max(0, d)
                nb0 = max(0, d // 512)
                spsum = ps.tile([128, WJ], F32, tag="sc", name="spsum")
                for nb in range(nb0, 2):
                    s0 = max(nb * 512, col0)
                    nc.tensor.matmul(
                        spsum[:, s0:(nb + 1) * 512],
                        kslc(h, i),
                        qslc(h, WJ * J + s0, WJ * J + (nb + 1) * 512),
                        start=True, stop=True)
                esb = sbe.tile([128, WJ], BF, tag="exp", name="esb")
                nc.scalar.activation(
                    esb[:, col0:WJ], spsum[:, col0:WJ],
                    mybir.ActivationFunctionType.Exp, scale=0.125)
                if d >= 0:
                    nc.vector.tensor_mul(
                        esb[:, d:d + 128], esb[:, d:d + 128], tri_sb)
                return esb

            def attn_back(J, h, i, ctx_ps, esb):
                d = 128 * i - WJ * J
                col0 = max(0, d)
                nb0 = max(0, d // 512)
                for nb in range(nb0, 2):
                    s0 = max(nb * 512, col0)
                    nc.tensor.matmul(
                        ctx_ps[:, s0:(nb + 1) * 512],
                        vaug[:, i, h, :],
                        esb[:, s0:(nb + 1) * 512],
                        start=(i == 0), stop=(i == 8 * J + 4 * nb + 3))

            def attn_norm(J, h, ctx_ps):
                den = sbo.tile([64, WJ], F32, tag="den", name="den")
                nc.vector.tensor_copy(den, ctx_ps[64:128, :])
                inv = sbo.tile([64, WJ], F32, tag="inv", name="inv")
                nc.vector.reciprocal_approx_fast(inv, den)
                h1tmp = (sbo.tile([64, WJ], BF, tag="h1tmp", name="h1tmp")
                         if h == 1 else None)
                dst = (ctxT01[0:64, WJ * J:WJ * (J + 1)], h1tmp,
                       ctxT2[:, WJ * J:WJ * (J + 1)])[h]
                nc.vector.tensor_mul(dst, ctx_ps[0:64, :], inv)
                if h == 1:
                    if o["shift_split"]:
                        for half in range(2):
                            c0 = half * 512
                            nc.sync.dma_start(
                                ctxT01[64:128, WJ * J + c0:WJ * J + c0 + 512],
                                h1tmp[:, c0:c0 + 512])
                    else:
                        nc.sync.dma_start(
                            ctxT01[64:128, WJ * J:WJ * (J + 1)], h1tmp)

            def out_proj(J, mt):
                ops = psc.tile([128, WJ], F32, tag="ctx", name="ops")
                for jj in range(2):
                    c0 = WJ * J + jj * 512
                    nc.tensor.matmul(
                        ops[:, jj * 512:(jj + 1) * 512],
                        wo01_sb[:, mt * 128:(mt + 1) * 128],
                        ctxT01[:, c0:c0 + 512], start=True, stop=False)
                    nc.tensor.matmul(
                        ops[:, jj * 512:(jj + 1) * 512],
                        wo2_sb[:, mt * 128:(mt + 1) * 128],
                        ctxT2[:, c0:c0 + 512], start=False, stop=True)
                if o["store_split"] or J == 1:
                    for jj in range(2):
                        osb = sbo.tile([128, 512], BF, tag="osb", name="osb")
                        if J == 1 and jj == 1:
                            # scalar engine is idle at the tail (exps done):
                            # split the PSUM evacuation across ACT + DVE
                            nc.scalar.copy(osb,
                                           ops[:, jj * 512:(jj + 1) * 512])
                        else:
                            nc.vector.tensor_copy(
                                osb, ops[:, jj * 512:(jj + 1) * 512])
                        nc.sync.dma_start(
                            outT[mt * 128:(mt + 1) * 128,
                                 WJ * J + jj * 512:WJ * J + (jj + 1) * 512],
                            osb)
                else:
                    osb = sbo.tile([128, WJ], BF, tag="osb", name="osb")
                    nc.vector.tensor_copy(osb, ops)
                    nc.sync.dma_start(
                        outT[mt * 128:(mt + 1) * 128, WJ * J:WJ * (J + 1)],
                        osb)

            def attention(J, h, fillers, smear=None, pops=1):
                # fillers: deque of closures to smear between kv-steps;
                # every `smear` steps, pop `pops` units
                smear = o["smear"] if smear is None else smear
                ctx_ps = psc.tile([128, WJ], F32, tag="ctx", name="ctx_ps")
                n = 8 * J + 8
                fifo = []
                for i in range(n):
                    esb = attn_front(J, h, i)
                    fifo.append((J, h, i, ctx_ps, esb))
                    if len(fifo) > o["defer"]:
                        attn_back(*fifo.pop(0))
                    if smear and i % smear == smear - 1:
                        for _ in range(pops):
                            if fillers:
                                fillers.pop(0)()
                for e in fifo:
                    attn_back(*e)
                attn_norm(J, h, ctx_ps)

            # ---- emission (v2-proven order); out_proj(0) is smeared into
            # attention(1,0) so its DMA-gated matmuls can't head-of-line
            # block the J1 score stream ----
            ho = o["head_order"]
            proj_qk(0, 0)
            proj_qk(1, 0)
            proj_v(0)
            # v_i is only needed at kv-step i -> feed the rest as fillers so
            # the exp stream starts ~10us earlier
            f0 = [lambda i=i: proj_v(i) for i in range(1, 8)]
            f0 += [lambda hf=hf: proj_qk(2, 0, half=hf) for hf in range(2)]
            f0 += [lambda i=i: proj_v(i) for i in range(8, NKV)]
            attention(0, ho[0], f0, smear=1, pops=2)
            while f0:
                f0.pop(0)()
            f1 = []
            for mt in range(3):
                f1 += [lambda mt=mt, hf=hf: proj_qk(mt, 1, half=hf)
                       for hf in range(2)]
            attention(0, ho[1], f1, smear=1)
            while f1:
                f1.pop(0)()
            attention(0, ho[2], [])
            # (att(0,h1) smear=1 pops all 6 ch1 proj halves in-chain; leaving
            # them to drain batched re-creates the measured pre-(0,2,0) gap)
            fillers = [lambda mt=mt: out_proj(0, mt) for mt in range(6)]
            attention(1, ho[0], fillers)
            while fillers:
                fillers.pop(0)()
            attention(1, ho[1], [])
            attention(1, ho[2], [])
            for mt in range(6):
                out_proj(1, mt)

    nc.compile()
    return nc


def shard_inputs(x, Wq, Wk, Wv, Wo):
    x = np.asarray(x, np.float32)
    tri = np.triu(np.ones((128, 128), np.float32))

    def bf(a):
        return np.ascontiguousarray(a).astype(ml_dtypes.bfloat16)

    in_maps = []
    for c in range(NCORES):
        b, g = c // 4, c % 4
        rs = slice(GH * g, GH * g + GH)
        Wq_g = np.asarray(Wq, np.float32)[rs]  # [192, 768]
        Wk_g = np.asarray(Wk, np.float32)[rs]
        # packed rows: [q0 q1 | k0 k1 | q2 k2] (64 rows each)
        wqk = np.concatenate(
            [Wq_g[0:128], Wk_g[0:128], Wq_g[128:192], Wk_g[128:192]],
            axis=0).T  # [768, 384]
        in_maps.append({
            "xT": bf(x[b].T),
            "wqk": bf(wqk),
            "wv": bf(np.asarray(Wv, np.float32)[rs].T),
            "wo": bf(np.asarray(Wo, np.float32)[:, rs].T),
            "tri": bf(tri),
        })
    return in_maps


def assemble(results, bo):
    out = np.zeros((B, S, D), np.float32)
    for c in range(NCORES):
        out[c // 4] += results[c]["outT"].astype(np.float32).T
    return out + np.asarray(bo, np.float32)[None, None, :]


_NC = None


def kernel(x, Wq, Wk, Wv, Wo, bo, **run_kwargs):
    global _NC
    if _NC is None:
        _NC = build()
    in_maps = shard_inputs(x, Wq, Wk, Wv, Wo)
    res = run_bass_kernel_spmd(_NC, in_maps, core_ids=list(range(NCORES)),
                               **run_kwargs)
    out = assemble(res.results, bo)
    kernel.last_results = res
    return out
